# revision 12
# baseline (speedup 1.0000x reference)
"""Trainium2 Bass kernel for nn_CorrOptDiMPUnique (DiMP correlation-filter
steepest-descent optimizer, 2 iterations).

Sharding: data-parallel over the S=8 sequences, one per NeuronCore.

Per core the math is restructured around a Gram matrix:

  Phi[k=(j,c), q]  : 3x3x256 feature windows over the 25x25 padded grid
  scoresT = Phi^T fT                      [625, 529]   (fwd conv, iter 1 only)
  G       = Phi^T Phi                     [625, 625]   (computed once)
  gradT   = Phi RT + reg fT               [2304, 529]  (wgrad)
  sgT     = Phi^T gradT = G RT + reg scoresT            (scores_grad, cheap)
  scoresT' = scoresT - step * alpha * sgT               (iter-2 fwd conv free)

Heavy matmuls run at 1 cycle/row: fp32r for the feature/filter path, fp16
for the wgrad/Gram path (phiT, RT, G). Filters are moved in k-major
[2304, 530] layout (host pre/post-transposes + pads nf to 530 so all
fp32r moving widths are even and >=256). Elementwise phases operate on
fused [125, 5*530] tiles to amortize DVE op overhead; DMAs are a few
large transfers ordered so compute starts immediately.
"""

import numpy as np

import concourse.bacc as bacc
import concourse.mybir as mybir
import concourse.tile as tile
from concourse.alu_op_type import AluOpType
from concourse.bass_utils import run_bass_kernel_spmd

F32 = mybir.dt.float32
F32R = mybir.dt.float32r
FP16 = mybir.dt.float16
BF16 = mybir.dt.bfloat16
U8 = mybir.dt.uint8
AF = mybir.ActivationFunctionType

S, C, H, W, FSZ = 8, 256, 23, 23, 3
HW = H * W                      # 529
NF = HW
NFP = 530                       # nf padded to even (f32r moving width rule)
PW = W + 2                      # 25: padded grid width
QP = PW * PW                    # 625 padded positions
KK = C * FSZ * FSZ              # 2304
NKC = KK // 128                 # 18 k-chunks
MARG = 26                       # margin so shifted window reads stay in bounds
FPW = MARG + QP + MARG + 1      # 678 feature-pad width (+1 for even G chunk)
NT = 5                          # q tiles
QT = 125                        # partitions per q tile
NW = NT * NFP                   # 2650: fused elementwise row width
NCH = [(0, 264), (264, 266)]    # nf chunks: >=256 and even (f32r rule)
GCH = [(0, 320), (320, 306)]    # q2 chunks for the Gram build (even widths)
NUM_ITER = 2
MIN_FILTER_REG = 1e-5
NUM_BINS = 10
BIN_DISP = 0.5


def _host_maps(w_label: np.ndarray, w_spatial: np.ndarray):
    """[625, 530] label map / half spatial weight / diag index, numpy only."""
    dH, dW = 2 * H - 1, 2 * W - 1
    d0 = np.arange(dH, dtype=np.float32) - (dH // 2)
    d1 = np.arange(dW, dtype=np.float32) - (dW // 2)
    dist = np.sqrt(d0[:, None] ** 2 + d1[None, :] ** 2)
    bin_diff = dist[None] / BIN_DISP - np.arange(
        NUM_BINS, dtype=np.float32)[:, None, None]
    main = np.maximum(1.0 - np.abs(bin_diff[:-1]), 0.0)
    last = np.clip(1.0 + bin_diff[-1:], 0.0, 1.0)
    bins = np.concatenate([main, last], axis=0)
    label_full = np.einsum("b,bhw->hw", w_label.astype(np.float32), bins)
    sw_full = np.einsum("b,bhw->hw", w_spatial.astype(np.float32), bins)

    # m[(y,x),(i,j)] = full[H-1-i+y, W-1-j+x]  (symmetric in hw<->nf)
    yy = np.arange(H)
    iy = (H - 1) - yy[None, :] + yy[:, None]          # [y, i]
    ix = iy                                           # W == H
    lm = label_full[iy[:, None, :, None], ix[None, :, None, :]].reshape(HW, HW)
    sm = sw_full[iy[:, None, :, None], ix[None, :, None, :]].reshape(HW, HW)
    label_pad = np.zeros((QP, NFP), np.float32)
    swh_pad = np.zeros((QP, NFP), np.float32)
    hwq = np.full((QP, 1), -1.0, np.float32)
    yy2, xx2 = np.meshgrid(np.arange(H), np.arange(W), indexing="ij")
    qidx = ((yy2 + 1) * PW + (xx2 + 1)).ravel()       # padded index of real hw
    label_pad[qidx, :NF] = lm
    swh_pad[qidx, :NF] = 0.5 * sm
    hwq[qidx, 0] = np.arange(HW, dtype=np.float32)
    return label_pad, swh_pad, hwq, qidx


def _delta(j):  # flat padded-grid shift for kernel tap j = dy*3+dx
    dy, dx = j // 3, j % 3
    return (dy - 1) * PW + (dx - 1)


def _win(fpad_ap, t, j, width=QT, base=0):
    """[128, width] window into the padded feature (tap shift j, q tile t)."""
    o = MARG + _delta(j) + QT * t + base
    return fpad_ap[:, o:o + width]


def build_nc():
    nc = bacc.Bacc(None, target_bir_lowering=False)
    w_in = nc.dram_tensor("w_in", (KK, NFP), F32, kind="ExternalInput")
    feat_in = nc.dram_tensor("feat_in", (C, HW), F32, kind="ExternalInput")
    phi_in = nc.dram_tensor("phi_in", (QP, KK), FP16, kind="ExternalInput")
    label_in = nc.dram_tensor("label_in", (QP, NFP), F32, kind="ExternalInput")
    swh_in = nc.dram_tensor("swh_in", (QP, NFP), F32, kind="ExternalInput")
    hwq_in = nc.dram_tensor("hwq_in", (QP, 1), F32, kind="ExternalInput")
    # scl_in[p,0] = -step_length, scl_in[p,1] = reg_weight (replicated rows)
    scl_in = nc.dram_tensor("scl_in", (128, 2), F32, kind="ExternalInput")
    f_out = nc.dram_tensor("f_out", (KK, NFP), F32, kind="ExternalOutput")

    def TS(t, n0=0, nw=NFP):  # fused-tile slice for q-tile t, nf range
        return slice(t * NFP + n0, t * NFP + n0 + nw)

    with tile.TileContext(nc) as tc:
        with tc.tile_pool(name="big", bufs=1) as big:
            # ---------- persistent tiles ----------
            fpad = [big.tile([128, FPW], F32R, name=f"fpad{c2}") for c2 in range(2)]
            phiT = [big.tile([QT, KK], FP16, name=f"phiT{t}") for t in range(NT)]
            G = [big.tile([QT, QP + 1], FP16, name=f"G{t}") for t in range(NT)]
            fTB = big.tile([128, NKC * NFP], F32R, name="fTB")
            gTB = big.tile([128, NKC * NFP], F32, name="gTB")
            scB = big.tile([QT, NW], F32, name="scB")
            sgB = big.tile([QT, NW], F32, name="sgB")
            RTB = big.tile([QT, NW], FP16, name="RTB")
            swmB = big.tile([QT, NW], F32, name="swmB")
            swhB = big.tile([QT, NW], F32, name="swhB")
            labB = big.tile([QT, NW], F32, name="labB")
            tmtB = big.tile([QT, NW], U8, name="tmtB")
            alphaS = big.tile([128, NFP], F32, name="alphaS")
            hwq_sb = big.tile([128, NT], F32, name="hwq_sb")
            scl = big.tile([128, 2], F32, name="scl")
            ones_col_f = big.tile([128, 1], F32, name="ones_col_f")
            ones_col_b = big.tile([128, 1], BF16, name="ones_col_b")
            ones_row_r = big.tile([1, 128], F32R, name="ones_row_r")

            def alpha_bc(p, k):  # alphaS broadcast over a chunk-index dim
                return alphaS[:p, :].unsqueeze(1).broadcast_to((p, k, NFP))

            # ---------- input DMAs (ordered by when data is needed) ----------
            with tc.tile_pool(name="sup", bufs=1) as sup:
                fstage = [sup.tile([128, FPW], F32, name=f"fstage{c2}")
                          for c2 in range(2)]
                iotac = sup.tile([128, NFP], F32, name="iotac")
                for c2 in range(2):
                    nc.vector.memset(fstage[c2], 0.0)
                    base = MARG + PW + 1
                    dst = fstage[c2][:, base:base + H * PW].rearrange(
                        "p (r v) -> p r v", v=PW)[:, :, :W]
                    nc.sync.dma_start(
                        out=dst,
                        in_=feat_in[c2 * 128:(c2 + 1) * 128, :].rearrange(
                            "p (h w) -> p h w", w=W))
                    nc.vector.tensor_copy(fpad[c2], fstage[c2])
                nc.sync.dma_start(out=scl, in_=scl_in[:, :])
                nc.sync.dma_start(
                    out=hwq_sb[:QT, :],
                    in_=hwq_in[:, :].rearrange("(t q) o -> q (t o)", t=NT))
                # filters: one big DMA into gTB staging, then round to f32r
                nc.sync.dma_start(
                    out=gTB.rearrange("p (kc n) -> p kc n", n=NFP),
                    in_=w_in[:, :].rearrange("(kc p) n -> p kc n", kc=NKC))
                for h in range(3):
                    w6 = 6 * NFP
                    nc.vector.tensor_copy(
                        fTB[:, h * w6:(h + 1) * w6], gTB[:, h * w6:(h + 1) * w6])
                nc.sync.dma_start(
                    out=labB.rearrange("q (t n) -> q t n", n=NFP),
                    in_=label_in[:, :].rearrange("(t q) n -> q t n", t=NT))
                nc.sync.dma_start(
                    out=swhB.rearrange("q (t n) -> q t n", n=NFP),
                    in_=swh_in[:, :].rearrange("(t q) n -> q t n", t=NT))
                for t in range(NT):
                    nc.sync.dma_start(
                        out=phiT[t], in_=phi_in[QT * t:QT * (t + 1), :])

                # masks + ones
                nc.gpsimd.iota(iotac, pattern=[[1, NFP]], base=0,
                               channel_multiplier=0,
                               allow_small_or_imprecise_dtypes=True)
                for t in range(NT):
                    nc.vector.tensor_scalar(
                        out=tmtB[:, TS(t)], in0=iotac[:QT, :],
                        scalar1=hwq_sb[:QT, t:t + 1], scalar2=None,
                        op0=AluOpType.is_equal)
                nc.vector.memset(ones_col_f, 1.0)
                nc.vector.tensor_copy(ones_col_b, ones_col_f)
                ones_row_f = sup.tile([1, 128], F32, name="ones_row_f")
                nc.vector.memset(ones_row_f, 1.0)
                nc.vector.tensor_copy(ones_row_r, ones_row_f)

            # ---------- main ----------
            with (
                tc.tile_pool(name="wrk", bufs=1) as wrk,
                tc.tile_pool(name="psmm", bufs=2, space="PSUM") as psmm,
                tc.tile_pool(name="psred", bufs=1, space="PSUM") as psred,
            ):
                # --- Gram matrix G = Phi^T Phi (f32r matmuls, fp16 store)
                # t1=0,1 before the fwd conv (while filters land); t1=2,3,4
                # after it, so they overlap the residual-phase DVE work.
                def build_G(t1):
                    pg = [psmm.tile([128, 512], F32, tag=f"a{gi}",
                                    name=f"pg{t1}_{gi}") for gi in range(2)]
                    for kc in range(NKC):
                        j, c2 = kc // 2, kc % 2
                        lhsT = _win(fpad[c2], t1, j)
                        for gi, (g0, gw) in enumerate(GCH):
                            nc.tensor.matmul(
                                pg[gi][:QT, :gw], lhsT,
                                _win(fpad[c2], 0, j, width=gw, base=g0),
                                start=(kc == 0), stop=(kc == NKC - 1))
                    for gi, (g0, gw) in enumerate(GCH):
                        nc.scalar.copy(
                            G[t1][:, g0:g0 + gw], pg[gi][:QT, :gw])

                for t1 in (0, 1):
                    build_G(t1)

                # --- forward conv iter 1: scoresT = Phi^T fT ---
                for t in range(NT):
                    ps = [psmm.tile([128, 512], F32, tag=f"a{ni}",
                                    name=f"pf{t}_{ni}") for ni in range(2)]
                    for kc in range(NKC):
                        j, c2 = kc // 2, kc % 2
                        lhsT = _win(fpad[c2], t, j)
                        for ni, (n0, nw) in enumerate(NCH):
                            nc.tensor.matmul(
                                ps[ni][:QT, :nw], lhsT,
                                fTB[:, kc * NFP + n0:kc * NFP + n0 + nw],
                                start=(kc == 0), stop=(kc == NKC - 1))
                    for ni, (n0, nw) in enumerate(NCH):
                        nc.scalar.copy(
                            scB[:, TS(t, n0, nw)], ps[ni][:QT, :nw])

                for t1 in (2, 3, 4):
                    build_G(t1)

                for it in range(NUM_ITER):
                    if it == 1:
                        # scoresT_2 = scoresT + alphaS*sgT  (alphaS = -step*a)
                        nc.gpsimd.tensor_mul(
                            sgB.rearrange("q (t n) -> q t n", n=NFP),
                            sgB.rearrange("q (t n) -> q t n", n=NFP),
                            alpha_bc(QT, NT))
                        nc.vector.tensor_add(scB, scB, sgB)

                    # --- residual: RT = swm * sw*(act - label), swm = mask*sw
                    act = wrk.tile([QT, NW], F32, tag="act")
                    sgn = wrk.tile([QT, NW], F32, tag="sgn")
                    nc.scalar.activation(out=act, in_=scB, func=AF.Relu)
                    nc.vector.copy_predicated(out=act, mask=tmtB, data=scB)
                    nc.scalar.activation(out=sgn, in_=scB, func=AF.Sign)
                    nc.vector.copy_predicated(
                        out=sgn, mask=tmtB,
                        data=ones_col_f[:QT, 0:1].broadcast_to((QT, NW)))
                    nc.gpsimd.tensor_sub(act, act, labB)
                    nc.vector.scalar_tensor_tensor(
                        out=swmB, in0=sgn, scalar=1.0, in1=swhB,
                        op0=AluOpType.add, op1=AluOpType.mult)
                    nc.vector.scalar_tensor_tensor(
                        out=act, in0=act, scalar=2.0, in1=swhB,
                        op0=AluOpType.mult, op1=AluOpType.mult)
                    nc.gpsimd.tensor_mul(RTB, swmB, act)

                    # --- sgT = G RT + reg*scoresT ; den += (swm*sgT)^2 ---
                    den_ps = [psred.tile([1, 512], F32, tag=f"den{ni}",
                                         name=f"den{it}_{ni}")
                              for ni in range(2)]
                    for t in range(NT):
                        ps = [psmm.tile([128, 512], F32, tag=f"a{ni}",
                                        name=f"psg{it}_{t}_{ni}")
                              for ni in range(2)]
                        for tp in range(NT):
                            lhsT = G[tp][:, QT * t:QT * t + QT]
                            for ni, (n0, nw) in enumerate(NCH):
                                nc.tensor.matmul(
                                    ps[ni][:QT, :nw], lhsT,
                                    RTB[:, TS(tp, n0, nw)],
                                    start=(tp == 0), stop=(tp == NT - 1))
                        for ni, (n0, nw) in enumerate(NCH):
                            nc.vector.scalar_tensor_tensor(
                                out=sgB[:, TS(t, n0, nw)],
                                in0=scB[:, TS(t, n0, nw)],
                                scalar=scl[:QT, 1:2], in1=ps[ni][:QT, :nw],
                                op0=AluOpType.mult, op1=AluOpType.add)
                    sgm = wrk.tile([QT, NW], F32, tag="act")   # reuse act buf
                    nc.vector.tensor_mul(sgm, swmB, sgB)
                    sq2 = wrk.tile([QT, NW], BF16, tag="sq2")
                    nc.scalar.activation(out=sq2, in_=sgm, func=AF.Square)
                    for t in range(NT):
                        for ni, (n0, nw) in enumerate(NCH):
                            nc.tensor.matmul(
                                den_ps[ni][:, :nw], ones_col_b[:QT, :],
                                sq2[:, TS(t, n0, nw)],
                                start=(t == 0), stop=(t == NT - 1))

                    # --- wgrad: gT = Phi RT + reg*fT ; num += gT^2 ---
                    num_ps = [psred.tile([1, 512], F32, tag=f"num{ni}",
                                         name=f"num{it}_{ni}")
                              for ni in range(2)]
                    for kc in range(NKC):
                        if it == 1 and kc % 6 == 0:
                            # fT += alphaS*gT for this 6-kc group (gpsimd,
                            # overlaps wgrad matmuls on PE / stt on DVE)
                            w0, w1 = kc * NFP, (kc + 6) * NFP
                            gv = gTB[:, w0:w1].rearrange(
                                "p (k n) -> p k n", n=NFP)
                            nc.gpsimd.tensor_mul(gv, gv, alpha_bc(128, 6))
                            nc.gpsimd.tensor_add(
                                fTB[:, w0:w1], fTB.bitcast(F32)[:, w0:w1],
                                gTB[:, w0:w1])
                        ps = [psmm.tile([128, 512], F32, tag=f"a{ni}",
                                        name=f"pw{it}_{kc}_{ni}")
                              for ni in range(2)]
                        for t in range(NT):
                            lhsT = phiT[t][:, kc * 128:(kc + 1) * 128]
                            for ni, (n0, nw) in enumerate(NCH):
                                nc.tensor.matmul(
                                    ps[ni][:, :nw], lhsT,
                                    RTB[:, TS(t, n0, nw)],
                                    start=(t == 0), stop=(t == NT - 1))
                        for ni, (n0, nw) in enumerate(NCH):
                            nc.vector.scalar_tensor_tensor(
                                out=gTB[:, kc * NFP + n0:kc * NFP + n0 + nw],
                                in0=fTB.bitcast(F32)[
                                    :, kc * NFP + n0:kc * NFP + n0 + nw],
                                scalar=scl[:, 1:2], in1=ps[ni][:, :nw],
                                op0=AluOpType.mult, op1=AluOpType.add)
                        sq = wrk.tile([128, NFP], BF16, tag="sq", bufs=2)
                        nc.scalar.activation(
                            out=sq, in_=gTB[:, kc * NFP:(kc + 1) * NFP],
                            func=AF.Square)
                        for ni, (n0, nw) in enumerate(NCH):
                            nc.tensor.matmul(
                                num_ps[ni][:, :nw], ones_col_b,
                                sq[:, n0:n0 + nw],
                                start=(kc == 0), stop=(kc == NKC - 1))

                    # --- alpha = -step * num / (den + reg*num) ---
                    alpha_r = wrk.tile([1, NFP], F32R, tag="alpha")
                    nsb = wrk.tile([1, NFP], F32, tag="nsb")
                    den2 = wrk.tile([1, NFP], F32, tag="dn2")
                    dn2r = wrk.tile([1, NFP], F32, tag="dn2r")
                    for ni, (n0, nw) in enumerate(NCH):
                        nc.scalar.copy(nsb[:, n0:n0 + nw], num_ps[ni][:, :nw])
                        nc.vector.scalar_tensor_tensor(
                            out=den2[:, n0:n0 + nw], in0=nsb[:, n0:n0 + nw],
                            scalar=scl[0:1, 1:2], in1=den_ps[ni][:, :nw],
                            op0=AluOpType.mult, op1=AluOpType.add)
                        nc.vector.tensor_scalar_max(
                            den2[:, n0:n0 + nw], den2[:, n0:n0 + nw], 1e-8)
                        nc.vector.reciprocal(
                            dn2r[:, n0:n0 + nw], den2[:, n0:n0 + nw])
                        nc.vector.scalar_tensor_tensor(
                            out=alpha_r[:, n0:n0 + nw], in0=nsb[:, n0:n0 + nw],
                            scalar=scl[0:1, 0:1], in1=dn2r[:, n0:n0 + nw],
                            op0=AluOpType.mult, op1=AluOpType.mult)
                    # broadcast alpha over partitions via K=1 ones matmul
                    for ni, (n0, nw) in enumerate(NCH):
                        pb = psmm.tile([128, 512], F32, tag=f"a{ni}",
                                       name=f"pb{it}_{ni}")
                        nc.tensor.matmul(
                            pb[:, :nw], ones_row_r, alpha_r[:, n0:n0 + nw],
                            start=True, stop=True)
                        nc.scalar.copy(alphaS[:, n0:n0 + nw], pb[:, :nw])

                # ---- final: f = fT + alphaS*gT -> DMA out (k-major) ----
                for h in range(6):
                    k0, k1 = h * 3, (h + 1) * 3
                    w0, w1 = k0 * NFP, k1 * NFP
                    eng = nc.vector if h % 2 == 0 else nc.gpsimd
                    gv = gTB[:, w0:w1].rearrange("p (k n) -> p k n", n=NFP)
                    eng.tensor_mul(gv, gv, alpha_bc(128, 3))
                    eng.tensor_add(
                        gTB[:, w0:w1], gTB[:, w0:w1],
                        fTB.bitcast(F32)[:, w0:w1])
                    nc.sync.dma_start(
                        out=f_out[k0 * 128:k1 * 128, :].rearrange(
                            "(kc p) n -> p kc n", p=128),
                        in_=gTB[:, w0:w1].rearrange(
                            "p (kc n) -> p kc n", n=NFP))

    nc.compile()
    return nc


_NC_CACHE = {}


def _get_nc():
    if "nc" not in _NC_CACHE:
        _NC_CACHE["nc"] = build_nc()
    return _NC_CACHE["nc"]


def make_in_maps(filter, feat, w_label, w_spatial, log_step_length, filter_reg):
    filter = np.asarray(filter)
    feat = np.asarray(feat)
    label_m, swh_m, hwq, qidx = _host_maps(
        np.asarray(w_label), np.asarray(w_spatial))
    step = float(np.exp(np.asarray(log_step_length)))
    reg = float(max(float(np.asarray(filter_reg)) ** 2, MIN_FILTER_REG ** 2))
    scl = np.tile(np.array([[-step, reg]], np.float32), (128, 1))

    in_maps = []
    for s in range(S):
        # filter [NF, C, 3, 3] -> k-major [KK, NFP] with k = (dy*3+dx)*256 + c
        w_kp = np.zeros((KK, NFP), np.float32)
        w_kp[:, :NF] = filter[s].reshape(NF, C, 9).transpose(2, 1, 0).reshape(KK, NF)
        feat_s = feat[0, s].reshape(C, HW).astype(np.float32)
        # phi windows [625, 2304] fp16: phi[q, j*C+c] = fpad[c, 26+q+delta(j)]
        fp = np.zeros((C, FPW), np.float32)
        fp[:, MARG + qidx] = feat_s
        phi = np.empty((QP, KK), np.float16)
        for j in range(9):
            o = MARG + _delta(j)
            phi[:, j * C:(j + 1) * C] = fp[:, o:o + QP].T.astype(np.float16)
        in_maps.append({
            "w_in": np.ascontiguousarray(w_kp),
            "feat_in": np.ascontiguousarray(feat_s),
            "phi_in": phi,
            "label_in": label_m,
            "swh_in": swh_m,
            "hwq_in": hwq,
            "scl_in": scl,
        })
    return in_maps


def postprocess(results):
    outs = []
    for s in range(S):
        f_kp = results[s]["f_out"].reshape(9, C, NFP)[:, :, :NF]
        outs.append(np.ascontiguousarray(
            f_kp.transpose(2, 1, 0)).reshape(NF, C, FSZ, FSZ))
    return np.stack(outs, axis=0).astype(np.float32)


def kernel(filter, feat, w_label, w_spatial, log_step_length, filter_reg):
    in_maps = make_in_maps(filter, feat, w_label, w_spatial,
                           log_step_length, filter_reg)
    nc = _get_nc()
    res = run_bass_kernel_spmd(nc, in_maps, core_ids=list(range(S)))
    return postprocess(res.results)
